# revision 8
# baseline (speedup 1.0000x reference)
"""Fused attention-with-relative-position-bias kernel for 8 TRN2 NeuronCores.

Problem: nn_AttentionHeader (B=2, S=1024, H=1024, NH=16, HD=64, MP=128).

Sharding: data-parallel over batch x tensor-parallel over heads.
core c (0..7): batch b = c//4, head-group hg = c%4 -> heads [4hg, 4hg+4),
i.e. columns [256hg, 256hg+256) of Wq/Wk/Wv. Each core's output slice is
disjoint, so no collectives are needed; the host re-assembles.

Positional-bias trick: einsum('ld,lrd->lr', q, dist_emb[dist]) factorizes as
P = q @ dist_emb.T  ([S,255]) followed by a Toeplitz gather
p(l,r) = clamp(l-r,-127,127)+127.  We compute, per 128-row l-tile,
Rext = q_tile @ de_ext  ([128,512]) where de_ext's columns are the
(reversed, edge-replicated) embeddings; write it to DRAM contiguously and
re-read with row stride 511 instead of 512 -> each successive row's window
shifts by one column = exactly the per-row diagonal gather. Edge clamping is
pre-baked into de_ext's replicated edge columns; only the l=0 row / r=0
column quirk needs explicit fix-ups.
"""

import numpy as np
import ml_dtypes
from contextlib import ExitStack

import concourse.bass as bass
import concourse.tile as tile
from concourse import bacc
from concourse import mybir
from concourse.bass_utils import run_bass_kernel_spmd
from concourse.masks import make_identity

B, S, H, NH, MP = 2, 1024, 1024, 16, 128
HD = H // NH           # 64
NCORES = 8
HPC = 4                # heads per core
CS = HPC * HD          # 256 weight columns per core
NLT = S // 128         # 8 l-tiles
F32 = mybir.dt.float32
BF16 = mybir.dt.bfloat16
BF = ml_dtypes.bfloat16

_CACHE = {}


def _build():
    nc = bacc.Bacc(target_bir_lowering=False)

    xT = nc.declare_dram_parameter("xT", [H, S], BF16, isOutput=False)        # hidden[b].T
    wq = nc.declare_dram_parameter("wq", [H, CS], BF16, isOutput=False)
    wk = nc.declare_dram_parameter("wk", [H, CS], BF16, isOutput=False)
    wv = nc.declare_dram_parameter("wv", [H, CS], BF16, isOutput=False)
    bq = nc.declare_dram_parameter("bq", [CS], F32, isOutput=False)
    bk = nc.declare_dram_parameter("bk", [CS], F32, isOutput=False)
    bv = nc.declare_dram_parameter("bv", [CS], F32, isOutput=False)
    de = nc.declare_dram_parameter("de", [128, 512], BF16, isOutput=False)    # de_ext x2
    out = nc.declare_dram_parameter("out", [HPC, S, HD], F32, isOutput=True)

    with tile.TileContext(nc) as tc, ExitStack() as ctx:
        singles = ctx.enter_context(tc.tile_pool(name="singles", bufs=1))

        # ---- constants / persistent tiles ----
        de_sb = singles.tile([128, 512], BF16, tag="de", name="de")
        nc.sync.dma_start(out=de_sb[:, :], in_=de[:, :])
        ident = singles.tile([HD, HD], F32, tag="ident", name="ident")
        make_identity(nc, ident[:, :])

        bvrow = singles.tile([128, CS], F32, tag="bvrow", name="bvrow")
        bv_bcast = bass.AP(tensor=bv[:].tensor, offset=0, ap=[[0, 128], [1, CS]])
        nc.sync.dma_start(out=bvrow[:, :], in_=bv_bcast)

        bq_sb = [singles.tile([128, 1], F32, tag=f"bq{m}", name=f"bq{m}") for m in range(2)]
        bk_sb = [singles.tile([128, 1], F32, tag=f"bk{m}", name=f"bk{m}") for m in range(2)]
        for m in range(2):
            nc.sync.dma_start(out=bq_sb[m][:, :],
                              in_=bq[m * 128:(m + 1) * 128].unsqueeze(1))
            nc.sync.dma_start(out=bk_sb[m][:, :],
                              in_=bk[m * 128:(m + 1) * 128].unsqueeze(1))

        xt_sb = [singles.tile([128, S], BF16, tag=f"xt{k}", name=f"xt{k}") for k in range(8)]
        wq_sb = [singles.tile([128, CS], BF16, tag=f"wq{k}", name=f"wq{k}") for k in range(8)]
        wk_sb = [singles.tile([128, CS], BF16, tag=f"wk{k}", name=f"wk{k}") for k in range(8)]
        wv_sb = [singles.tile([128, CS], BF16, tag=f"wv{k}", name=f"wv{k}") for k in range(8)]
        for k in range(8):
            r = slice(k * 128, (k + 1) * 128)
            nc.sync.dma_start(out=xt_sb[k][:, :], in_=xT[r, :])
            nc.sync.dma_start(out=wq_sb[k][:, :], in_=wq[r, :])
            nc.sync.dma_start(out=wk_sb[k][:, :], in_=wk[r, :])
            nc.sync.dma_start(out=wv_sb[k][:, :], in_=wv[r, :])

        qT_sb = [singles.tile([128, S], BF16, tag=f"qT{m}", name=f"qT{m}") for m in range(2)]
        kT_sb = [singles.tile([128, S], BF16, tag=f"kT{m}", name=f"kT{m}") for m in range(2)]
        v_sb = [singles.tile([128, CS], BF16, tag=f"v{t}", name=f"v{t}") for t in range(8)]

        # ---- projections ----
        with tc.tile_pool(name="psum_proj", bufs=2, space="PSUM") as pproj:
            # Q^T, K^T: out[c, s] = sum_h W[h, c] * xT[h, s]
            for w_sb, b_sb, o_sb in ((wq_sb, bq_sb, qT_sb), (wk_sb, bk_sb, kT_sb)):
                for m in range(2):
                    for nh2 in range(2):
                        ps = pproj.tile([128, 512], F32, tag="pq", name="pq")
                        for k in range(8):
                            nc.tensor.matmul(
                                ps[:, :],
                                w_sb[k][:, m * 128:(m + 1) * 128],
                                xt_sb[k][:, nh2 * 512:(nh2 + 1) * 512],
                                start=(k == 0), stop=(k == 7))
                        nc.vector.tensor_scalar_add(
                            o_sb[m][:, nh2 * 512:(nh2 + 1) * 512],
                            ps[:, :], b_sb[m][:, :])
            # V: out[s, c] = sum_h xT[h, s] * Wv[h, c]
            for t in range(8):
                ps = pproj.tile([128, CS], F32, tag="pv", name="pv")
                for k in range(8):
                    nc.tensor.matmul(
                        ps[:, :],
                        xt_sb[k][:, t * 128:(t + 1) * 128],
                        wv_sb[k][:, :],
                        start=(k == 0), stop=(k == 7))
                nc.vector.tensor_tensor(v_sb[t][:, :], ps[:, :], bvrow[:, :],
                                        op=mybir.AluOpType.add)

        # ---- per-head attention ----
        psum_r = ctx.enter_context(tc.tile_pool(name="psum_r", bufs=1, space="PSUM"))
        psum_s = ctx.enter_context(tc.tile_pool(name="psum_s", bufs=2, space="PSUM"))
        psum_ct = ctx.enter_context(tc.tile_pool(name="psum_ct", bufs=2, space="PSUM"))
        psum_c2 = ctx.enter_context(tc.tile_pool(name="psum_c2", bufs=1, space="PSUM"))
        dram = ctx.enter_context(tc.tile_pool(name="dram", bufs=4, space="DRAM"))
        sb = ctx.enter_context(tc.tile_pool(name="sb", bufs=3))
        sbp = ctx.enter_context(tc.tile_pool(name="sbp", bufs=2))

        SCR = 128 * 511 + 512   # flat DRAM scratch, covers strided re-read

        for h in range(HPC):
            m, r0 = h // 2, (h % 2) * HD
            qT_h = qT_sb[m][r0:r0 + HD, :]
            kT_h = kT_sb[m][r0:r0 + HD, :]

            rows_t = sbp.tile([128, NLT], F32, tag="rows", name="rows")
            probsT = [sbp.tile([128, S], BF16, tag=f"pT{rj}", name=f"pT{rj}") for rj in range(8)]

            for lt in range(NLT):
                l0 = lt * 128
                # Rext = q_lt @ de_ext
                pR = psum_r.tile([128, 512], F32, tag="pR", name="pR")
                nc.tensor.matmul(pR[:, :], qT_h[:, l0:l0 + 128], de_sb[r0:r0 + HD, :],
                                 start=True, stop=True)
                rext = sb.tile([128, 512], BF16, tag="rext", name="rext")
                nc.scalar.copy(rext[:, :], pR[:, :])
                cols = sb.tile([128, 3], F32, tag="cols", name="cols")
                nc.vector.tensor_copy(cols[:, :], pR[:, 128:383:127])
                scr = dram.tile([SCR], BF16, tag="scr", name="scr")
                nc.sync.dma_start(
                    out=scr[0:65536].rearrange("(p f) -> p f", f=512),
                    in_=rext[:, :])

                # scores = q_lt @ K^T
                pS = psum_s.tile([128, S], F32, tag="pS", name="pS")
                nc.tensor.matmul(pS[:, 0:512], qT_h[:, l0:l0 + 128],
                                 kT_h[:, 0:512], start=True, stop=True)
                nc.tensor.matmul(pS[:, 512:1024], qT_h[:, l0:l0 + 128],
                                 kT_h[:, 512:1024], start=True, stop=True)

                # skewed band re-read (row stride 511 = per-row shift)
                rj_lo, rj_hi = max(lt - 1, 0), min(lt + 1, NLT - 1)
                rb = rj_lo * 128
                W = (rj_hi - rj_lo + 1) * 128
                s0p = rb - l0 + 255
                band = sb.tile([128, 384], BF16, tag="band", name="band")
                band_src = scr[s0p:s0p + 511 * 128].rearrange(
                    "(p f) -> p f", f=511)[:, 0:W]
                nc.sync.dma_start(out=band[:, 0:W], in_=band_src)

                # bias-add pieces -> scores_sb (f32)
                sc = sb.tile([128, S], F32, tag="sc", name="sc")
                nc.vector.tensor_tensor(sc[:, rb:rb + W], pS[:, rb:rb + W],
                                        band[:, 0:W], op=mybir.AluOpType.add)
                if rb > 0:
                    nc.vector.tensor_scalar_add(sc[:, 0:rb], pS[:, 0:rb],
                                                cols[:, 0:1])
                if rb + W < S:
                    nc.vector.tensor_scalar_add(sc[:, rb + W:S], pS[:, rb + W:S],
                                                cols[:, 2:3])
                nc.vector.tensor_scalar_add(sc[:, 0:1], pS[:, 0:1],
                                            cols[:, 1:2])
                if lt == 0:
                    nc.vector.tensor_scalar_add(sc[0:1, :], pS[0:1, :],
                                                cols[0:1, 1:2])

                # softmax numerator (+ fused row-sum), 1/sqrt(64) folded in
                probs = sb.tile([128, S], BF16, tag="probs", name="probs")
                nc.scalar.activation(probs[:, :], sc[:, :],
                                     func=mybir.ActivationFunctionType.Exp,
                                     scale=0.125,
                                     accum_out=rows_t[:, lt:lt + 1])

                for rj in range(8):
                    nc.sync.dma_start_transpose(
                        out=probsT[rj][:, l0:l0 + 128],
                        in_=probs[:, rj * 128:(rj + 1) * 128])

            recip_t = sbp.tile([128, NLT], F32, tag="recip", name="recip")
            nc.vector.reciprocal(recip_t[:, :], rows_t[:, :])

            # ctx^T[d, l] = sum_r V[r, d] * probsT[r, l]
            ctxT = sbp.tile([HD, S], F32, tag="ctxT", name="ctxT")
            for nh2 in range(2):
                pC = psum_ct.tile([HD, 512], F32, tag="pC", name="pC")
                for rj in range(8):
                    nc.tensor.matmul(
                        pC[:, :],
                        v_sb[rj][:, h * HD:(h + 1) * HD],
                        probsT[rj][:, nh2 * 512:(nh2 + 1) * 512],
                        start=(rj == 0), stop=(rj == 7))
                nc.vector.tensor_copy(ctxT[:, nh2 * 512:(nh2 + 1) * 512], pC[:, :])

            for lt in range(NLT):
                l0 = lt * 128
                pX = psum_c2.tile([128, HD], F32, tag="pX", name="pX")
                nc.tensor.transpose(pX[:, :], ctxT[:, l0:l0 + 128], ident[:, :])
                cx = sb.tile([128, HD], F32, tag="cx", name="cx")
                nc.vector.tensor_scalar_mul(cx[:, :], pX[:, :],
                                            recip_t[:, lt:lt + 1])
                nc.sync.dma_start(out=out[h, l0:l0 + 128, :], in_=cx[:, :])

    nc.finalize()
    return nc


def _prep_inputs(hidden_states, Wq, bq, Wk, bk, Wv, bv, dist_emb):
    idx = 254 - np.clip(np.arange(512) - 128, 0, 254)
    de1 = dist_emb[idx].T.astype(BF)
    de_ext = np.ascontiguousarray(np.vstack([de1, de1]))  # [128, 512]
    xTs = [np.ascontiguousarray(hidden_states[b].T).astype(BF) for b in range(B)]
    in_maps = []
    for c in range(NCORES):
        b, hg = c // 4, c % 4
        cs = slice(hg * CS, (hg + 1) * CS)
        in_maps.append({
            "xT": xTs[b],
            "wq": np.ascontiguousarray(Wq[:, cs]).astype(BF),
            "wk": np.ascontiguousarray(Wk[:, cs]).astype(BF),
            "wv": np.ascontiguousarray(Wv[:, cs]).astype(BF),
            "bq": np.ascontiguousarray(bq[cs]).astype(np.float32),
            "bk": np.ascontiguousarray(bk[cs]).astype(np.float32),
            "bv": np.ascontiguousarray(bv[cs]).astype(np.float32),
            "de": de_ext,
        })
    return in_maps


def _assemble(results):
    full = np.empty((B, S, H), np.float32)
    for c in range(NCORES):
        b, hg = c // 4, c % 4
        o = np.asarray(results[c]["out"], np.float32)       # [4, S, 64]
        full[b, :, hg * CS:(hg + 1) * CS] = o.transpose(1, 0, 2).reshape(S, CS)
    return full


def kernel(hidden_states, Wq, bq, Wk, bk, Wv, bv, dist_emb, _trace=False):
    if "nc" not in _CACHE:
        _CACHE["nc"] = _build()
    in_maps = _prep_inputs(np.asarray(hidden_states, np.float32),
                           np.asarray(Wq, np.float32), np.asarray(bq, np.float32),
                           np.asarray(Wk, np.float32), np.asarray(bk, np.float32),
                           np.asarray(Wv, np.float32), np.asarray(bv, np.float32),
                           np.asarray(dist_emb, np.float32))
    if _trace:
        import concourse.bass_utils as _bu
        _bu.upload_artifacts = lambda d: f"local:{d}"
    res = run_bass_kernel_spmd(_CACHE["nc"], in_maps, core_ids=list(range(NCORES)),
                               trace=_trace)
    if _trace:
        kernel.last_exec_time_ns = res.exec_time_ns
    return _assemble(res.results)


kernel.last_exec_time_ns = None
